# revision 33
# baseline (speedup 1.0000x reference)
"""AttentionPool Trainium2 kernel: 8-core data-parallel Bass/Tile implementation.

Reference computation (per batch b of 32, S=2048, D=1024):
    xn = LayerNorm(x[b])                      # over D, eps 1e-5
    h = tanh(xn @ W1 + b1)
    scores = h @ W2 + b2                      # [S]
    w = softmax(scores)
    out[b] = sum_s w[s] * x[b, s, :]

Strategy: batch axis sharded over 8 cores (4 batches each). Per core/batch:
  - x loaded once as plain f32 on the sync HWDGE ring (no SWDGE anywhere:
    DMA-xbar transposes serialize against outstanding SWDGE transfers on
    HW, which repeatedly stalled the pipeline). The f32 copy stays in
    SBUF and feeds the pooling matmuls as f32r (free bitcast).
  - LN stats via bn_stats/bn_aggr + Newton rsqrt (DVE); normalize emits
    fp8e4 directly, split between ACT (Identity w/ per-partition
    scale=rstd, bias=-mu*rstd) and DVE (tensor_scalar) to balance load.
  - fp8 xn staged to a per-quarter DRAM scratch (sync), then read back
    via DMA-xbar transpose at 2-byte granularity: fp8 PAIRS per partition
    = exactly the [K,2,N] moving layout DoubleRow contracts over.
  - matmul1 in fp8 DoubleRow (half the K-tiles of bf16), weights packed
    on host in (super-tile, partition, plane) order, scaled by 32 for
    e4m3; the 1/32 undo rides the tanh activation's scale.
  - tanh+c2 on ACT over [128, 1024] PSUM tiles; scores matmul in bf16,
    all 4 chunks accumulated in ONE PSUM bank at partitions 0/32/64/96
    (tile_position col-tiling), emitted one e-group late so ACT latency
    never stalls the in-order PE queue.
  - b2 dropped (softmax shift-invariance); Z comes from the exp ops' ACT
    accumulator partials bounced through DRAM to partitions 0 and 32.
  - pooling matmuls in f32r against the SBUF-resident x copy, both
    d-halves per subtile back-to-back into rows 0/32 of one PSUM bank
    (shared stationary -> LDWEIGHTS dedup).
Pipelining: each batch's phase3 runs as two chunk-pair passes (batch 0:
four single-chunk passes for fast ramp); phase1(b+1) is EMITTED
interleaved into phase3(b)'s e-loops (hooks) so the ACT/DVE/Sync queues
carry next-batch prep between this batch's tanh/score work, and each
quarter's transposes trail only their own load/write. Pooling of b-1 is
emitted at the first pass's e0 so the softmax scatter stays hidden.
Host-side prep folds ln_gamma into W1 and ln_beta@W1+b1 into c2.
"""
import sys
import os

sys.path.insert(0, '/opt/trn_rl_repo')

import numpy as np

import concourse.bass as bass
import concourse.tile as tile
from concourse import bacc, mybir
from concourse.bass_utils import run_bass_kernel_spmd

P = 128
D = 1024
S = 2048
B = 32
NCORES = 8
BLOC = B // NCORES            # batches per core
ROWS = BLOC * S               # 8192 rows per core
SUBT = S // P                 # 16 subtiles per batch
NG = 4                        # subtiles per stats/normalize group (= quarter)
CHUNK = 512                   # matmul moving free dim (output cols)
NCHUNK = S // CHUNK           # 4 chunks per batch
ET = D // P                   # 8 e-tiles

FP8 = True                    # matmul1 via fp8 DoubleRow
W1SCALE = 32.0                # host scales W1 by this; undone in tanh's scale
KT = 4 if FP8 else 8          # contraction super-tiles for matmul1
NPT = 4 if FP8 else 8         # transposed partition-tiles per batch

f32 = mybir.dt.float32
f32r = mybir.dt.float32r
bf16 = mybir.dt.bfloat16
fp8 = mybir.dt.float8e4
AF = mybir.ActivationFunctionType
ALU = mybir.AluOpType
DR = mybir.MatmulPerfMode.DoubleRow
XDT = fp8 if FP8 else bf16    # staged-xn dtype


def build_nc():
    nc = bacc.Bacc("TRN2", target_bir_lowering=False, num_devices=NCORES)

    # f32r so the plain HWDGE load feeds the f32r pooling matmuls directly
    # (same bytes as f32; LN reads go through a bitcast-f32 view)
    x = nc.dram_tensor("x", [ROWS, D], f32r, kind="ExternalInput")
    if FP8:
        w1p = nc.dram_tensor("w1p", [P, KT, 2, ET, P], fp8,
                             kind="ExternalInput")
    else:
        w1p = nc.dram_tensor("w1p", [P, KT, ET, P], bf16,
                             kind="ExternalInput")
    c2v = nc.dram_tensor("c2v", [D], f32, kind="ExternalInput")
    w2v = nc.dram_tensor("w2v", [D], bf16, kind="ExternalInput")
    out = nc.dram_tensor("out", [BLOC, D], f32, kind="ExternalOutput")

    with tile.TileContext(nc) as tc:
        with (
            tc.tile_pool(name="consts", bufs=1) as consts,
            tc.tile_pool(name="xa", bufs=2) as xap,        # [128,16,1024] f32
            tc.tile_pool(name="stats", bufs=8) as statp,
            tc.tile_pool(name="xnst", bufs=2) as xnst,     # [128,4,1024] fp8
            tc.tile_pool(name="xt", bufs=2) as xtp,        # [128,NPT,2048] bf16
            tc.tile_pool(name="ht", bufs=3) as htp,        # [128,<=1024] bf16
            tc.tile_pool(name="sc", bufs=3) as scp,        # small score tiles
            tc.tile_pool(name="ob", bufs=2) as obp,
            tc.tile_pool(name="psmm", bufs=2, space="PSUM") as psmm,  # 2 banks
            tc.tile_pool(name="pssc", bufs=1, space="PSUM") as pssc,  # 1 bank
            tc.tile_pool(name="pspl", bufs=2, space="PSUM") as pspl,  # 2 banks
            tc.tile_pool(name="dram", bufs=8, space="DRAM") as dramp,
        ):
            # ---- constants ----
            w1_sb = consts.tile(list(w1p.shape), fp8 if FP8 else bf16)
            nc.scalar.dma_start(w1_sb, w1p.ap())
            c2_sb = consts.tile([P, ET], f32)
            nc.scalar.dma_start(c2_sb, c2v.ap().rearrange("(t p) -> p t", p=P))
            w2_sb = consts.tile([P, ET], bf16)
            nc.scalar.dma_start(w2_sb, w2v.ap().rearrange("(t p) -> p t", p=P))
            x3 = x.ap().rearrange("(b t p) d -> b t p d", b=BLOC, p=P)

            def p1_load(b, xa, g, split=False):
                """Quarter g's x load (plain f32r, scalar HWDGE ring --
                keeps plain S2M traffic off the sync ring that carries the
                xbar transposes; sharing that ring corrupted them on HW)."""
                t0 = NG * g
                if split:
                    for s2 in range(0, NG, 2):
                        nc.scalar.dma_start(
                            xa[:, t0 + s2:t0 + s2 + 2, :],
                            x3[b, t0 + s2:t0 + s2 + 2].rearrange(
                                "t p d -> p t d"))
                else:
                    nc.scalar.dma_start(
                        xa[:, t0:t0 + NG, :],
                        x3[b, t0:t0 + NG].rearrange("t p d -> p t d"))

            def p1_chain(b, xa, xtt, g):
                """Quarter g: LN stats + Newton + normalize -> fp8 scratch +
                transposes."""
                t0 = NG * g
                xaf = xa.bitcast(f32)
                mv = statp.tile([P, NG, 2], f32, tag="mv")
                for s in range(NG):
                    st = statp.tile([P, 2, 6], f32, tag="bnst")
                    nc.vector.bn_stats(st[:, 0, :], xaf[:, t0 + s, 0:512])
                    nc.vector.bn_stats(st[:, 1, :], xaf[:, t0 + s, 512:1024])
                    nc.vector.bn_aggr(mv[:, s, :], st)
                # rstd = rsqrt(var+eps): quake seed + 2 Newton steps (DVE)
                var = statp.tile([P, NG], f32, tag="var")
                nc.vector.tensor_scalar(out=var, in0=mv[:, :, 1],
                                        scalar1=1e-5, scalar2=0.5,
                                        op0=ALU.add, op1=ALU.mult)
                y = statp.tile([P, NG], f32, tag="y")
                yi = y.bitcast(mybir.dt.int32)
                vi = var.bitcast(mybir.dt.int32)
                nc.vector.tensor_scalar(out=yi, in0=vi, scalar1=0x800000,
                                        scalar2=None, op0=ALU.add)
                nc.vector.tensor_scalar(out=yi, in0=yi, scalar1=1,
                                        scalar2=None,
                                        op0=ALU.logical_shift_right)
                nc.vector.tensor_scalar(out=yi, in0=yi, scalar1=-1,
                                        scalar2=0x5f3759df,
                                        op0=ALU.mult, op1=ALU.add)
                tny = statp.tile([P, NG], f32, tag="tny")
                for _ in range(2):
                    nc.vector.tensor_tensor(tny, y, y, ALU.mult)
                    nc.vector.tensor_tensor(tny, tny, var, ALU.mult)
                    nc.vector.tensor_scalar(out=tny, in0=tny, scalar1=-1.0,
                                            scalar2=1.5,
                                            op0=ALU.mult, op1=ALU.add)
                    nc.vector.tensor_tensor(y, y, tny, ALU.mult)
                # mb = -mu * rstd (ACT normalize bias)
                mb = statp.tile([P, NG], f32, tag="mb")
                nc.vector.tensor_tensor(mb, mv[:, :, 0], y, ALU.mult)
                nc.vector.tensor_scalar(out=mb, in0=mb, scalar1=-1.0,
                                        scalar2=None, op0=ALU.mult)
                xnb = xnst.tile([P, NG, D], XDT, tag="xnst")
                for s in range(NG):
                    if s % 2 == 0:
                        nc.scalar.activation(xnb[:, s, :],
                                             xaf[:, t0 + s, :], AF.Identity,
                                             scale=y[:, s:s + 1],
                                             bias=mb[:, s:s + 1])
                    else:
                        nc.vector.tensor_scalar(out=xnb[:, s, :],
                                                in0=xaf[:, t0 + s, :],
                                                scalar1=mv[:, s, 0:1],
                                                scalar2=y[:, s:s + 1],
                                                op0=ALU.subtract,
                                                op1=ALU.mult)
                scr_q = dramp.tile([CHUNK, D], XDT, tag="scratch")
                scrT = scr_q.bitcast(bf16)        # [512, D//2] pair view
                nc.sync.dma_start(
                    scr_q.rearrange("(t p) d -> t p d", p=P).rearrange(
                        "t p d -> p t d"), xnb)
                for t in range(NPT):
                    nc.sync.dma_start_transpose(
                        xtt[:, t, g * CHUNK:(g + 1) * CHUNK],
                        scrT[:, t * P:(t + 1) * P])

            def emit_pool_mms(pl0, pl1, epk, xa, c=None):
                """Pooling matmuls (f32r), subtiles of chunk c (or all 16).
                Both d-halves per subtile back-to-back (shared stationary
                epk column -> LDWEIGHTS dedup) into two partition-0 PSUM
                banks (f32r matmuls may only target partition 0)."""
                rng = range(4 * c, 4 * c + 4) if c is not None else range(SUBT)
                for t in rng:
                    s = (t - 4 * c) if c is not None else t
                    nc.tensor.matmul(pl0, epk[:, s:s + 1],
                                     xa[:, t, 0:512],
                                     start=(t == 0), stop=(t == SUBT - 1))
                    nc.tensor.matmul(pl1, epk[:, s:s + 1],
                                     xa[:, t, 512:1024],
                                     start=(t == 0), stop=(t == SUBT - 1))

            def z_chain(zc, zb):
                """1/Z at partition 0 from the 4 per-chunk exp accumulator
                partials (partitions 0/32/64/96) via a tiny DRAM bounce."""
                nc.scalar.dma_start(
                    zb, zc.rearrange("(a b) f -> a b f", b=32)[:, 0, :])
                z4 = scp.tile([1, NCHUNK], f32, tag="z4")
                zt = scp.tile([1, 1], f32, tag="zt")
                rz = scp.tile([1, 1], f32, tag="rz")
                nc.scalar.dma_start(z4, zb.rearrange("(a c) -> a c", a=1))
                nc.vector.tensor_reduce(zt, z4, axis=mybir.AxisListType.X,
                                        op=ALU.add)
                nc.vector.reciprocal(rz, zt)
                return rz

            def pool_store(b, pl0, pl1, rz):
                """Scaled copies from the two partition-0 PSUM banks + the
                two out stores."""
                ob0 = obp.tile([1, 512], f32, tag="ob0")
                nc.scalar.activation(ob0, pl0, AF.Copy, scale=rz[0:1, 0:1])
                nc.sync.dma_start(out.ap()[b:b + 1, 0:512], ob0)
                ob1 = obp.tile([1, 512], f32, tag="ob1")
                nc.scalar.activation(ob1, pl1, AF.Copy, scale=rz[0:1, 0:1])
                nc.sync.dma_start(out.ap()[b:b + 1, 512:1024], ob1)

            def phase4(b, epk_f, zc, zb, xa):
                """Batch-level pooling for a non-last batch."""
                rz = z_chain(zc, zb)
                epk = scp.tile([P, SUBT], f32r, tag="epk")
                nc.vector.tensor_copy(epk, epk_f)
                pl0 = pspl.tile([1, 512], f32, tag="pspl")
                pl1 = pspl.tile([1, 512], f32, tag="pspl")
                emit_pool_mms(pl0, pl1, epk, xa)
                pool_store(b, pl0, pl1, rz)

            def phase3_pass(b, xa, xtt, group, sc_ps, sc_first, sc_last,
                            hooks):
                """matmul1 + tanh + scores for one chunk-group (a tuple of
                chunks sharing one PSUM tile). `hooks[e]` emits next-batch
                phase1 pieces / previous-batch pooling inside the e-loop."""
                f8 = xtt.bitcast(fp8) if FP8 else None   # [128,KT,4096]
                hts = [None] * ET

                def rhs(t, c):
                    if FP8:
                        return f8[:, t, c * 2 * CHUNK:(c + 1) * 2 * CHUNK] \
                            .rearrange("p (s two) -> p two s", two=2)
                    return xtt[:, t, c * CHUNK:(c + 1) * CHUNK]

                def lhs(t, e):
                    if FP8:
                        return w1_sb[:, t, :, e, :]
                    return w1_sb[:, t, e, :]

                def emit_sc(e):
                    for j, c in enumerate(group):
                        nc.tensor.matmul(
                            sc_ps[32 * c:32 * c + 1, :], w2_sb[:, e:e + 1],
                            hts[e][:, j * CHUNK:(j + 1) * CHUNK],
                            start=(e == 0 and sc_first),
                            stop=(e == ET - 1 and sc_last),
                            tile_position=(0, 32 * c))

                pm = DR if FP8 else None
                tanh_scale = (1.0 / W1SCALE) if FP8 else 1.0
                w = len(group) * CHUNK
                for e in range(ET):
                    ps = psmm.tile([P, w], f32, tag="mm")
                    for t in range(KT):
                        for j, c in enumerate(group):
                            nc.tensor.matmul(
                                ps[:, j * CHUNK:(j + 1) * CHUNK],
                                lhs(t, e), rhs(t, c),
                                start=(t == 0), stop=(t == KT - 1),
                                perf_mode=pm)
                    hti = htp.tile([P, w], bf16, tag="ht")
                    nc.scalar.activation(hti, ps, AF.Tanh,
                                         bias=c2_sb[:, e:e + 1],
                                         scale=tanh_scale)
                    hts[e] = hti
                    if e >= 1:
                        emit_sc(e - 1)
                    for h in hooks.get(e, []):
                        h()
                emit_sc(ET - 1)

            def phase3(b, xa, xtt, prev, nxt):
                """Full phase3 as chunk-group passes with interleaved hooks
                for phase4(b-1) and phase1(b+1); exp + scatter at the end
                (+ inline pooling for the last batch)."""
                last = (b == BLOC - 1)
                sc_ps = pssc.tile([P, CHUNK], f32, tag="pssc")
                if b == 0:
                    passes = [(c,) for c in range(NCHUNK)]
                else:
                    passes = [(0, 1), (2, 3)]

                # distribute hooks over (pass, e) slots
                hooks = [dict() for _ in passes]
                if prev is not None:
                    hooks[0].setdefault(0, []).append(
                        lambda: phase4(*prev))
                if nxt is not None:
                    nxa, nxtt = nxt
                    ne = len(passes) * ET
                    for g in range(NG):
                        ld = (g * 2) * ne // 8 + 1
                        ch = (g * 2 + 1) * ne // 8 + 1
                        hooks[ld // ET].setdefault(ld % ET, []).append(
                            (lambda gg: lambda: p1_load(b + 1, nxa, gg))(g))
                        hooks[ch // ET].setdefault(ch % ET, []).append(
                            (lambda gg: lambda: p1_chain(b + 1, nxa, nxtt,
                                                         gg))(g))

                for pi, grp in enumerate(passes):
                    phase3_pass(b, xa, xtt, grp, sc_ps,
                                sc_first=True, sc_last=True,
                                hooks=hooks[pi])

                ec = scp.tile([P, CHUNK], f32, tag="ec")
                zc = scp.tile([P, 1], f32, tag="zc")
                eb = dramp.tile([S], f32, tag="eb")
                zb = dramp.tile([NCHUNK], f32, tag="zb")
                if not last:
                    for c in range(NCHUNK):
                        nc.scalar.activation(ec[32 * c:32 * c + 1, :],
                                             sc_ps[32 * c:32 * c + 1, :],
                                             AF.Exp,
                                             accum_out=zc[32 * c:32 * c + 1, :])
                    nc.scalar.dma_start(
                        eb.rearrange("(c j) -> c j", c=NCHUNK),
                        ec.rearrange("(a b) f -> a b f", b=32)[:, 0, :])
                    epk_f = scp.tile([P, SUBT], f32, tag="epkf")
                    nc.scalar.dma_start(
                        epk_f, eb.rearrange("(t p) -> p t", p=P))
                    return (b, epk_f, zc, zb, xa)

                # last batch: per-chunk scatter + inline pooling
                pl0 = pspl.tile([1, 512], f32, tag="pspl")
                pl1 = pspl.tile([1, 512], f32, tag="pspl")
                for c in range(NCHUNK):
                    nc.scalar.activation(ec[32 * c:32 * c + 1, :],
                                         sc_ps[32 * c:32 * c + 1, :],
                                         AF.Exp,
                                         accum_out=zc[32 * c:32 * c + 1, :])
                    nc.scalar.dma_start(eb[c * CHUNK:(c + 1) * CHUNK],
                                        ec[32 * c:32 * c + 1, :])
                    epk_f = scp.tile([P, NCHUNK], f32, tag="epkf")
                    nc.scalar.dma_start(
                        epk_f,
                        eb[c * CHUNK:(c + 1) * CHUNK].rearrange(
                            "(t p) -> p t", p=P))
                    epk = scp.tile([P, NCHUNK], f32r, tag="epk")
                    nc.vector.tensor_copy(epk, epk_f)
                    emit_pool_mms(pl0, pl1, epk, xa, c=c)
                rz = z_chain(zc, zb)
                pool_store(b, pl0, pl1, rz)
                return None

            # batch 0 prologue: all loads (split for ramp), then chains
            tiles = [(xap.tile([P, SUBT, D], f32r, tag="xa", name=f"xa{b}"),
                      xtp.tile([P, NPT, S], bf16, tag="xt", name=f"xt{b}"))
                     for b in range(2)]

            def get_tiles(b):
                if b < 2:
                    return tiles[b]
                return (xap.tile([P, SUBT, D], f32r, tag="xa", name=f"xa{b}"),
                        xtp.tile([P, NPT, S], bf16, tag="xt", name=f"xt{b}"))

            xa0, xtt0 = tiles[0]
            for g in range(NG):
                p1_load(0, xa0, g, split=True)
            for g in range(NG):
                p1_chain(0, xa0, xtt0, g)

            prev = None
            cur = tiles[0]
            nxt = tiles[1]
            for b in range(BLOC):
                xa, xtt = cur
                prev = phase3(b, xa, xtt, prev,
                              nxt if b < BLOC - 1 else None)
                cur = nxt
                if b + 2 < BLOC:
                    nxt = get_tiles(b + 2)
            assert prev is None

    nc.compile()
    return nc


_NC_CACHE = {}


def _get_nc():
    if "nc" not in _NC_CACHE:
        _NC_CACHE["nc"] = build_nc()
    return _NC_CACHE["nc"]


def _prep_host(ln_gamma, ln_beta, W1, b1, W2, b2):
    import ml_dtypes
    W1g = (np.asarray(ln_gamma, np.float32)[:, None]
           * np.asarray(W1, np.float32))
    if FP8:
        # pack rows in DoubleRow (super-tile, partition, plane) order:
        # d = t*256 + p*2 + i  ->  arr[p, t, i, e8, e128]
        W1s = (W1g * W1SCALE).astype(ml_dtypes.float8_e4m3)
        W1pk = np.ascontiguousarray(
            W1s.reshape(KT, P, 2, ET, P).transpose(1, 0, 2, 3, 4))
    else:
        # d = t*128 + p  ->  arr[p, t, e8, e128]
        W1s = W1g.astype(ml_dtypes.bfloat16)
        W1pk = np.ascontiguousarray(
            W1s.reshape(KT, P, ET, P).transpose(1, 0, 2, 3))
    c2 = (np.asarray(ln_beta, np.float32) @ np.asarray(W1, np.float32)
          + np.asarray(b1, np.float32))
    w2v = np.ascontiguousarray(
        np.asarray(W2, np.float32)[:, 0]).astype(ml_dtypes.bfloat16)
    return W1pk, np.ascontiguousarray(c2), w2v


def run_cores(inputs, trace=False, **kw):
    x = np.asarray(inputs["x"], np.float32)
    W1pk, c2, w2v = _prep_host(inputs["ln_gamma"], inputs["ln_beta"],
                               inputs["W1"], inputs["b1"],
                               inputs["W2"], inputs["b2"])
    nc = _get_nc()
    in_maps = []
    for c in range(NCORES):
        shard = np.ascontiguousarray(
            x[c * BLOC:(c + 1) * BLOC].reshape(ROWS, D))
        in_maps.append(dict(x=shard, w1p=W1pk, c2v=c2, w2v=w2v))
    res = run_bass_kernel_spmd(nc, in_maps, core_ids=list(range(NCORES)),
                               trace=trace, **kw)
    full = np.concatenate([res.results[c]["out"] for c in range(NCORES)],
                          axis=0)
    return full, res


def kernel(**inputs) -> np.ndarray:
    out, _ = run_cores(inputs, trace=False)
    return out.astype(np.float32)


# revision 34
# speedup vs baseline: 1.0710x; 1.0710x over previous
"""AttentionPool Trainium2 kernel: 8-core data-parallel Bass/Tile implementation.

Reference computation (per batch b of 32, S=2048, D=1024):
    xn = LayerNorm(x[b])                      # over D, eps 1e-5
    h = tanh(xn @ W1 + b1)
    scores = h @ W2 + b2                      # [S]
    w = softmax(scores)
    out[b] = sum_s w[s] * x[b, s, :]

Strategy: batch axis sharded over 8 cores (4 batches each). Per core/batch:
  - x loaded once as plain f32 on the sync HWDGE ring (no SWDGE anywhere:
    DMA-xbar transposes serialize against outstanding SWDGE transfers on
    HW, which repeatedly stalled the pipeline). The f32 copy stays in
    SBUF and feeds the pooling matmuls as f32r (free bitcast).
  - LN stats via bn_stats/bn_aggr + Newton rsqrt (DVE); normalize emits
    fp8e4 directly, split between ACT (Identity w/ per-partition
    scale=rstd, bias=-mu*rstd) and DVE (tensor_scalar) to balance load.
  - fp8 xn staged to a per-quarter DRAM scratch (sync), then read back
    via DMA-xbar transpose at 2-byte granularity: fp8 PAIRS per partition
    = exactly the [K,2,N] moving layout DoubleRow contracts over.
  - matmul1 in fp8 DoubleRow (half the K-tiles of bf16), weights packed
    on host in (super-tile, partition, plane) order, scaled by 32 for
    e4m3; the 1/32 undo rides the tanh activation's scale.
  - tanh+c2 on ACT over [128, 1024] PSUM tiles; scores matmul in bf16,
    all 4 chunks accumulated in ONE PSUM bank at partitions 0/32/64/96
    (tile_position col-tiling), emitted one e-group late so ACT latency
    never stalls the in-order PE queue.
  - b2 dropped (softmax shift-invariance); Z comes from the exp ops' ACT
    accumulator partials bounced through DRAM to partitions 0 and 32.
  - pooling matmuls in f32r against the SBUF-resident x copy, both
    d-halves per subtile back-to-back into rows 0/32 of one PSUM bank
    (shared stationary -> LDWEIGHTS dedup).
Pipelining: each batch's phase3 runs as two chunk-pair passes (batch 0:
four single-chunk passes for fast ramp); phase1(b+1) is EMITTED
interleaved into phase3(b)'s e-loops (hooks) so the ACT/DVE/Sync queues
carry next-batch prep between this batch's tanh/score work, and each
quarter's transposes trail only their own load/write. Pooling of b-1 is
emitted at the first pass's e0 so the softmax scatter stays hidden.
Host-side prep folds ln_gamma into W1 and ln_beta@W1+b1 into c2.
"""
import sys
import os

sys.path.insert(0, '/opt/trn_rl_repo')

import numpy as np

import concourse.bass as bass
import concourse.tile as tile
from concourse import bacc, mybir
from concourse.bass_utils import run_bass_kernel_spmd

P = 128
D = 1024
S = 2048
B = 32
NCORES = 8
BLOC = B // NCORES            # batches per core
ROWS = BLOC * S               # 8192 rows per core
SUBT = S // P                 # 16 subtiles per batch
NG = 4                        # subtiles per stats/normalize group (= quarter)
CHUNK = 512                   # matmul moving free dim (output cols)
NCHUNK = S // CHUNK           # 4 chunks per batch
ET = D // P                   # 8 e-tiles

FP8 = True                    # matmul1 via fp8 DoubleRow
W1SCALE = 32.0                # host scales W1 by this; undone in tanh's scale
KT = 4 if FP8 else 8          # contraction super-tiles for matmul1
NPT = 4 if FP8 else 8         # transposed partition-tiles per batch

f32 = mybir.dt.float32
f32r = mybir.dt.float32r
bf16 = mybir.dt.bfloat16
fp8 = mybir.dt.float8e4
AF = mybir.ActivationFunctionType
ALU = mybir.AluOpType
DR = mybir.MatmulPerfMode.DoubleRow
XDT = fp8 if FP8 else bf16    # staged-xn dtype


def build_nc():
    nc = bacc.Bacc("TRN2", target_bir_lowering=False, num_devices=NCORES)

    # f32r so the plain HWDGE load feeds the f32r pooling matmuls directly
    # (same bytes as f32; LN reads go through a bitcast-f32 view)
    x = nc.dram_tensor("x", [ROWS, D], f32r, kind="ExternalInput")
    if FP8:
        w1p = nc.dram_tensor("w1p", [P, KT, 2, ET, P], fp8,
                             kind="ExternalInput")
    else:
        w1p = nc.dram_tensor("w1p", [P, KT, ET, P], bf16,
                             kind="ExternalInput")
    c2v = nc.dram_tensor("c2v", [D], f32, kind="ExternalInput")
    w2v = nc.dram_tensor("w2v", [D], bf16, kind="ExternalInput")
    out = nc.dram_tensor("out", [BLOC, D], f32, kind="ExternalOutput")

    with tile.TileContext(nc) as tc:
        with (
            tc.tile_pool(name="consts", bufs=1) as consts,
            tc.tile_pool(name="xa", bufs=2) as xap,        # [128,16,1024] f32
            tc.tile_pool(name="stats", bufs=8) as statp,
            tc.tile_pool(name="xnst", bufs=2) as xnst,     # [128,4,1024] fp8
            tc.tile_pool(name="xt", bufs=2) as xtp,        # [128,NPT,2048] bf16
            tc.tile_pool(name="ht", bufs=3) as htp,        # [128,<=1024] bf16
            tc.tile_pool(name="sc", bufs=3) as scp,        # small score tiles
            tc.tile_pool(name="ob", bufs=2) as obp,
            tc.tile_pool(name="psmm", bufs=2, space="PSUM") as psmm,  # 2 banks
            tc.tile_pool(name="pssc", bufs=1, space="PSUM") as pssc,  # 1 bank
            tc.tile_pool(name="pspl", bufs=2, space="PSUM") as pspl,  # 2 banks
            tc.tile_pool(name="dram", bufs=8, space="DRAM") as dramp,
        ):
            # ---- constants ----
            w1_sb = consts.tile(list(w1p.shape), fp8 if FP8 else bf16)
            nc.scalar.dma_start(w1_sb, w1p.ap())
            c2_sb = consts.tile([P, ET], f32)
            nc.scalar.dma_start(c2_sb, c2v.ap().rearrange("(t p) -> p t", p=P))
            w2_sb = consts.tile([P, ET], bf16)
            nc.scalar.dma_start(w2_sb, w2v.ap().rearrange("(t p) -> p t", p=P))
            x3 = x.ap().rearrange("(b t p) d -> b t p d", b=BLOC, p=P)

            def p1_load(b, xa, g, split=False):
                """Quarter g's x load (plain f32r, gpsimd SWDGE ring).

                Keeps plain S2M traffic off the sync ring that carries the
                xbar transposes (sharing that ring corrupted them on HW),
                and off the ACT queue (load WAR waits head-of-line-block
                tanh there). The SWDGE ring carries ONLY these loads, so
                the fp8 scratch writes (sync) never queue behind them."""
                t0 = NG * g
                if split:
                    for s2 in range(0, NG, 2):
                        nc.gpsimd.dma_start(
                            xa[:, t0 + s2:t0 + s2 + 2, :],
                            x3[b, t0 + s2:t0 + s2 + 2].rearrange(
                                "t p d -> p t d"))
                else:
                    nc.gpsimd.dma_start(
                        xa[:, t0:t0 + NG, :],
                        x3[b, t0:t0 + NG].rearrange("t p d -> p t d"))

            def p1_chain(b, xa, xtt, g):
                """Quarter g: LN stats + Newton + normalize -> fp8 scratch +
                transposes."""
                t0 = NG * g
                xaf = xa.bitcast(f32)
                mv = statp.tile([P, NG, 2], f32, tag="mv")
                for s in range(NG):
                    st = statp.tile([P, 2, 6], f32, tag="bnst")
                    nc.vector.bn_stats(st[:, 0, :], xaf[:, t0 + s, 0:512])
                    nc.vector.bn_stats(st[:, 1, :], xaf[:, t0 + s, 512:1024])
                    nc.vector.bn_aggr(mv[:, s, :], st)
                # rstd = rsqrt(var+eps): quake seed + 2 Newton steps (DVE)
                var = statp.tile([P, NG], f32, tag="var")
                nc.vector.tensor_scalar(out=var, in0=mv[:, :, 1],
                                        scalar1=1e-5, scalar2=0.5,
                                        op0=ALU.add, op1=ALU.mult)
                y = statp.tile([P, NG], f32, tag="y")
                yi = y.bitcast(mybir.dt.int32)
                vi = var.bitcast(mybir.dt.int32)
                nc.vector.tensor_scalar(out=yi, in0=vi, scalar1=0x800000,
                                        scalar2=None, op0=ALU.add)
                nc.vector.tensor_scalar(out=yi, in0=yi, scalar1=1,
                                        scalar2=None,
                                        op0=ALU.logical_shift_right)
                nc.vector.tensor_scalar(out=yi, in0=yi, scalar1=-1,
                                        scalar2=0x5f3759df,
                                        op0=ALU.mult, op1=ALU.add)
                tny = statp.tile([P, NG], f32, tag="tny")
                for _ in range(2):
                    nc.vector.tensor_tensor(tny, y, y, ALU.mult)
                    nc.vector.tensor_tensor(tny, tny, var, ALU.mult)
                    nc.vector.tensor_scalar(out=tny, in0=tny, scalar1=-1.0,
                                            scalar2=1.5,
                                            op0=ALU.mult, op1=ALU.add)
                    nc.vector.tensor_tensor(y, y, tny, ALU.mult)
                # mb = -mu * rstd (ACT normalize bias)
                mb = statp.tile([P, NG], f32, tag="mb")
                nc.vector.tensor_tensor(mb, mv[:, :, 0], y, ALU.mult)
                nc.vector.tensor_scalar(out=mb, in0=mb, scalar1=-1.0,
                                        scalar2=None, op0=ALU.mult)
                xnb = xnst.tile([P, NG, D], XDT, tag="xnst")
                for s in range(NG):
                    if s % 2 == 0:
                        nc.scalar.activation(xnb[:, s, :],
                                             xaf[:, t0 + s, :], AF.Identity,
                                             scale=y[:, s:s + 1],
                                             bias=mb[:, s:s + 1])
                    else:
                        nc.vector.tensor_scalar(out=xnb[:, s, :],
                                                in0=xaf[:, t0 + s, :],
                                                scalar1=mv[:, s, 0:1],
                                                scalar2=y[:, s:s + 1],
                                                op0=ALU.subtract,
                                                op1=ALU.mult)
                scr_q = dramp.tile([CHUNK, D], XDT, tag="scratch")
                scrT = scr_q.bitcast(bf16)        # [512, D//2] pair view
                nc.sync.dma_start(
                    scr_q.rearrange("(t p) d -> t p d", p=P).rearrange(
                        "t p d -> p t d"), xnb)
                for t in range(NPT):
                    nc.sync.dma_start_transpose(
                        xtt[:, t, g * CHUNK:(g + 1) * CHUNK],
                        scrT[:, t * P:(t + 1) * P])

            def emit_pool_mms(pl0, pl1, epk, xa, c=None):
                """Pooling matmuls (f32r), subtiles of chunk c (or all 16).
                Both d-halves per subtile back-to-back (shared stationary
                epk column -> LDWEIGHTS dedup) into two partition-0 PSUM
                banks (f32r matmuls may only target partition 0)."""
                rng = range(4 * c, 4 * c + 4) if c is not None else range(SUBT)
                for t in rng:
                    s = (t - 4 * c) if c is not None else t
                    nc.tensor.matmul(pl0, epk[:, s:s + 1],
                                     xa[:, t, 0:512],
                                     start=(t == 0), stop=(t == SUBT - 1))
                    nc.tensor.matmul(pl1, epk[:, s:s + 1],
                                     xa[:, t, 512:1024],
                                     start=(t == 0), stop=(t == SUBT - 1))

            def z_chain(zc, zb):
                """1/Z at partition 0 from the 4 per-chunk exp accumulator
                partials (partitions 0/32/64/96) via a tiny DRAM bounce."""
                nc.scalar.dma_start(
                    zb, zc.rearrange("(a b) f -> a b f", b=32)[:, 0, :])
                z4 = scp.tile([1, NCHUNK], f32, tag="z4")
                zt = scp.tile([1, 1], f32, tag="zt")
                rz = scp.tile([1, 1], f32, tag="rz")
                nc.scalar.dma_start(z4, zb.rearrange("(a c) -> a c", a=1))
                nc.vector.tensor_reduce(zt, z4, axis=mybir.AxisListType.X,
                                        op=ALU.add)
                nc.vector.reciprocal(rz, zt)
                return rz

            def pool_store(b, pl0, pl1, rz):
                """Scaled copies from the two partition-0 PSUM banks + the
                two out stores."""
                ob0 = obp.tile([1, 512], f32, tag="ob0")
                nc.scalar.activation(ob0, pl0, AF.Copy, scale=rz[0:1, 0:1])
                nc.sync.dma_start(out.ap()[b:b + 1, 0:512], ob0)
                ob1 = obp.tile([1, 512], f32, tag="ob1")
                nc.scalar.activation(ob1, pl1, AF.Copy, scale=rz[0:1, 0:1])
                nc.sync.dma_start(out.ap()[b:b + 1, 512:1024], ob1)

            def phase4(b, epk_f, zc, zb, xa):
                """Batch-level pooling for a non-last batch."""
                rz = z_chain(zc, zb)
                epk = scp.tile([P, SUBT], f32r, tag="epk")
                nc.vector.tensor_copy(epk, epk_f)
                pl0 = pspl.tile([1, 512], f32, tag="pspl")
                pl1 = pspl.tile([1, 512], f32, tag="pspl")
                emit_pool_mms(pl0, pl1, epk, xa)
                pool_store(b, pl0, pl1, rz)

            def phase3_pass(b, xa, xtt, group, sc_ps, sc_first, sc_last,
                            hooks):
                """matmul1 + tanh + scores for one chunk-group (a tuple of
                chunks sharing one PSUM tile). `hooks[e]` emits next-batch
                phase1 pieces / previous-batch pooling inside the e-loop."""
                f8 = xtt.bitcast(fp8) if FP8 else None   # [128,KT,4096]
                hts = [None] * ET

                def rhs(t, c):
                    if FP8:
                        return f8[:, t, c * 2 * CHUNK:(c + 1) * 2 * CHUNK] \
                            .rearrange("p (s two) -> p two s", two=2)
                    return xtt[:, t, c * CHUNK:(c + 1) * CHUNK]

                def lhs(t, e):
                    if FP8:
                        return w1_sb[:, t, :, e, :]
                    return w1_sb[:, t, e, :]

                def emit_sc(e):
                    for j, c in enumerate(group):
                        nc.tensor.matmul(
                            sc_ps[32 * c:32 * c + 1, :], w2_sb[:, e:e + 1],
                            hts[e][:, j * CHUNK:(j + 1) * CHUNK],
                            start=(e == 0 and sc_first),
                            stop=(e == ET - 1 and sc_last),
                            tile_position=(0, 32 * c))

                pm = DR if FP8 else None
                tanh_scale = (1.0 / W1SCALE) if FP8 else 1.0
                w = len(group) * CHUNK
                for e in range(ET):
                    ps = psmm.tile([P, w], f32, tag="mm")
                    for t in range(KT):
                        for j, c in enumerate(group):
                            nc.tensor.matmul(
                                ps[:, j * CHUNK:(j + 1) * CHUNK],
                                lhs(t, e), rhs(t, c),
                                start=(t == 0), stop=(t == KT - 1),
                                perf_mode=pm)
                    hti = htp.tile([P, w], bf16, tag="ht")
                    nc.scalar.activation(hti, ps, AF.Tanh,
                                         bias=c2_sb[:, e:e + 1],
                                         scale=tanh_scale)
                    hts[e] = hti
                    if e >= 1:
                        emit_sc(e - 1)
                    for h in hooks.get(e, []):
                        h()
                emit_sc(ET - 1)

            def phase3(b, xa, xtt, prev, nxt):
                """Full phase3 as chunk-group passes with interleaved hooks
                for phase4(b-1) and phase1(b+1); exp + scatter at the end
                (+ inline pooling for the last batch)."""
                last = (b == BLOC - 1)
                sc_ps = pssc.tile([P, CHUNK], f32, tag="pssc")
                if b == 0:
                    passes = [(c,) for c in range(NCHUNK)]
                else:
                    passes = [(0, 1), (2, 3)]

                # distribute hooks over (pass, e) slots
                hooks = [dict() for _ in passes]
                if prev is not None:
                    hooks[0].setdefault(0, []).append(
                        lambda: phase4(*prev))
                if nxt is not None:
                    nxa, nxtt = nxt
                    ne = len(passes) * ET
                    for g in range(NG):
                        ld = (g * 2) * ne // 8 + 1
                        ch = (g * 2 + 1) * ne // 8 + 1
                        hooks[ld // ET].setdefault(ld % ET, []).append(
                            (lambda gg: lambda: p1_load(b + 1, nxa, gg))(g))
                        hooks[ch // ET].setdefault(ch % ET, []).append(
                            (lambda gg: lambda: p1_chain(b + 1, nxa, nxtt,
                                                         gg))(g))

                for pi, grp in enumerate(passes):
                    phase3_pass(b, xa, xtt, grp, sc_ps,
                                sc_first=True, sc_last=True,
                                hooks=hooks[pi])

                ec = scp.tile([P, CHUNK], f32, tag="ec")
                zc = scp.tile([P, 1], f32, tag="zc")
                eb = dramp.tile([S], f32, tag="eb")
                zb = dramp.tile([NCHUNK], f32, tag="zb")
                if not last:
                    for c in range(NCHUNK):
                        nc.scalar.activation(ec[32 * c:32 * c + 1, :],
                                             sc_ps[32 * c:32 * c + 1, :],
                                             AF.Exp,
                                             accum_out=zc[32 * c:32 * c + 1, :])
                    nc.scalar.dma_start(
                        eb.rearrange("(c j) -> c j", c=NCHUNK),
                        ec.rearrange("(a b) f -> a b f", b=32)[:, 0, :])
                    epk_f = scp.tile([P, SUBT], f32, tag="epkf")
                    nc.scalar.dma_start(
                        epk_f, eb.rearrange("(t p) -> p t", p=P))
                    return (b, epk_f, zc, zb, xa)

                # last batch: per-chunk scatter + inline pooling
                pl0 = pspl.tile([1, 512], f32, tag="pspl")
                pl1 = pspl.tile([1, 512], f32, tag="pspl")
                for c in range(NCHUNK):
                    nc.scalar.activation(ec[32 * c:32 * c + 1, :],
                                         sc_ps[32 * c:32 * c + 1, :],
                                         AF.Exp,
                                         accum_out=zc[32 * c:32 * c + 1, :])
                    nc.scalar.dma_start(eb[c * CHUNK:(c + 1) * CHUNK],
                                        ec[32 * c:32 * c + 1, :])
                    epk_f = scp.tile([P, NCHUNK], f32, tag="epkf")
                    nc.scalar.dma_start(
                        epk_f,
                        eb[c * CHUNK:(c + 1) * CHUNK].rearrange(
                            "(t p) -> p t", p=P))
                    epk = scp.tile([P, NCHUNK], f32r, tag="epk")
                    nc.vector.tensor_copy(epk, epk_f)
                    emit_pool_mms(pl0, pl1, epk, xa, c=c)
                rz = z_chain(zc, zb)
                pool_store(b, pl0, pl1, rz)
                return None

            # batch 0 prologue: all loads (split for ramp), then chains
            tiles = [(xap.tile([P, SUBT, D], f32r, tag="xa", name=f"xa{b}"),
                      xtp.tile([P, NPT, S], bf16, tag="xt", name=f"xt{b}"))
                     for b in range(2)]

            def get_tiles(b):
                if b < 2:
                    return tiles[b]
                return (xap.tile([P, SUBT, D], f32r, tag="xa", name=f"xa{b}"),
                        xtp.tile([P, NPT, S], bf16, tag="xt", name=f"xt{b}"))

            xa0, xtt0 = tiles[0]
            for g in range(NG):
                p1_load(0, xa0, g, split=True)
            for g in range(NG):
                p1_chain(0, xa0, xtt0, g)

            prev = None
            cur = tiles[0]
            nxt = tiles[1]
            for b in range(BLOC):
                xa, xtt = cur
                prev = phase3(b, xa, xtt, prev,
                              nxt if b < BLOC - 1 else None)
                cur = nxt
                if b + 2 < BLOC:
                    nxt = get_tiles(b + 2)
            assert prev is None

    nc.compile()
    return nc


_NC_CACHE = {}


def _get_nc():
    if "nc" not in _NC_CACHE:
        _NC_CACHE["nc"] = build_nc()
    return _NC_CACHE["nc"]


def _prep_host(ln_gamma, ln_beta, W1, b1, W2, b2):
    import ml_dtypes
    W1g = (np.asarray(ln_gamma, np.float32)[:, None]
           * np.asarray(W1, np.float32))
    if FP8:
        # pack rows in DoubleRow (super-tile, partition, plane) order:
        # d = t*256 + p*2 + i  ->  arr[p, t, i, e8, e128]
        W1s = (W1g * W1SCALE).astype(ml_dtypes.float8_e4m3)
        W1pk = np.ascontiguousarray(
            W1s.reshape(KT, P, 2, ET, P).transpose(1, 0, 2, 3, 4))
    else:
        # d = t*128 + p  ->  arr[p, t, e8, e128]
        W1s = W1g.astype(ml_dtypes.bfloat16)
        W1pk = np.ascontiguousarray(
            W1s.reshape(KT, P, ET, P).transpose(1, 0, 2, 3))
    c2 = (np.asarray(ln_beta, np.float32) @ np.asarray(W1, np.float32)
          + np.asarray(b1, np.float32))
    w2v = np.ascontiguousarray(
        np.asarray(W2, np.float32)[:, 0]).astype(ml_dtypes.bfloat16)
    return W1pk, np.ascontiguousarray(c2), w2v


def run_cores(inputs, trace=False, **kw):
    x = np.asarray(inputs["x"], np.float32)
    W1pk, c2, w2v = _prep_host(inputs["ln_gamma"], inputs["ln_beta"],
                               inputs["W1"], inputs["b1"],
                               inputs["W2"], inputs["b2"])
    nc = _get_nc()
    in_maps = []
    for c in range(NCORES):
        shard = np.ascontiguousarray(
            x[c * BLOC:(c + 1) * BLOC].reshape(ROWS, D))
        in_maps.append(dict(x=shard, w1p=W1pk, c2v=c2, w2v=w2v))
    res = run_bass_kernel_spmd(nc, in_maps, core_ids=list(range(NCORES)),
                               trace=trace, **kw)
    full = np.concatenate([res.results[c]["out"] for c in range(NCORES)],
                          axis=0)
    return full, res


def kernel(**inputs) -> np.ndarray:
    out, _ = run_cores(inputs, trace=False)
    return out.astype(np.float32)
